# revision 30
# baseline (speedup 1.0000x reference)
# Banked (MoE top-2) feedforward on 8 TRN2 NeuronCores.
#
# Strategy (expert-parallel with hidden-dim splitting for load balance):
#   - Router (tiny: [T,1024]@[1024,16]) runs on host with jax-CPU, matching the
#     reference's einsum/softmax/top_k numerics so bank selection is identical.
#   - Work unit = half a bank: (bank e, half h) owns 16 of the 32 hidden
#     subtiles. Each half-piece computes HT = relu(W1h^T X^T + b1h) over its
#     16 hidden subtiles and a PARTIAL YT = W2h^T HT (fc2 contracting only its
#     half of D_HIDDEN, all 8 output subtiles). The host sums the two halves
#     and adds b2 during the combine, so no bias pass is needed for fc2.
#   - 32 half-pieces are sorted by token count and grouped 8-at-a-time into 4
#     per-core slots with graded capacities (cap = max count in group), which
#     cuts padded columns from ~1074 to ~1050 per core vs whole-bank slots.
#   - All matmuls in fp16 (full-rate PE, 2-byte weight loads); psum accumulates
#     fp32; y written back as fp16 (halves writeback traffic; host combines in
#     fp32). fp8 was measured at 4.3e-2 rel err -- over the gate -- so fp16.
#   - Head optimization: slot 0's first chunk is small so the first matmul only
#     waits for ~0.5MB of DMA; per-hm chunk interleave keeps the w1 stream rate
#     sustainable (~150GB/s) instead of bursting.

import os
import sys
import types

import numpy as np

# The device kernel runs through jax's axon PJRT backend. If the caller's
# environment pins JAX_PLATFORMS (e.g. to "cpu" for the reference), extend it
# so the axon backend stays reachable. Must happen before jax's first import.
_jp = os.environ.get("JAX_PLATFORMS", "")
if _jp and "axon" not in _jp.split(","):
    os.environ["JAX_PLATFORMS"] = _jp + ",axon"

D_MODEL = 1024
D_HIDDEN = 4096
NUM_BANKS = 16
NUM_SELECTED = 2
N_CORES = 8
P = 128
KD = D_MODEL // P     # 8  contraction subtiles for fc1
MH = D_HIDDEN // P    # 32 hidden subtiles total
_SPLIT = int(os.environ.get("BANKED_SPLIT", "2"))  # hidden pieces per bank
HH = MH // _SPLIT     # hidden subtiles per piece
MO = D_MODEL // P     # 8  output subtiles for fc2
N_SLOTS = NUM_BANKS * _SPLIT // N_CORES  # pieces per core

_PROFILE = bool(int(os.environ.get("BANKED_PROFILE", "0")))
_ASC = bool(int(os.environ.get("BANKED_ASC", "0")))      # slot order ascending
_WARM = int(os.environ.get("BANKED_WARM", "9"))          # warmup matmuls
_W2PF = int(os.environ.get("BANKED_W2PF", "0"))          # w2 tiles pre-issued
_XGATE = bool(int(os.environ.get("BANKED_XGATE", "0")))  # gate x_{s+1} on fc1_s
_XLATE = bool(int(os.environ.get("BANKED_XLATE", "0")))  # x_{s+1} after fc1
_FSMALL = int(os.environ.get("BANKED_FSMALL", "320"))    # small first chunk
_W1BUFS = int(os.environ.get("BANKED_W1BUFS", "6"))      # w1 prefetch depth
_W1ABUFS = int(os.environ.get("BANKED_W1ABUFS", "6"))    # slot-0 w1 depth

last_exec_time_ns = None
last_results = None


def _install_ntff_hook():
    """bass_utils' axon trace path imports antenv.axon_hooks, which this image
    lacks; shim it and register the ctypes-based NTFF hook."""
    if "antenv.axon_hooks" in sys.modules:
        return
    mod = types.ModuleType("antenv.axon_hooks")
    mod._hook = None
    mod.set_axon_ntff_profile_hook = lambda h: setattr(mod, "_hook", h)
    mod.get_axon_ntff_profile_hook = lambda: mod._hook
    sys.modules["antenv.axon_hooks"] = mod
    try:
        from trn_agent_boot.trn_boot import _ntff_profile_via_ctypes

        mod.set_axon_ntff_profile_hook(
            _ntff_profile_via_ctypes("/opt/axon/libaxon_pjrt.so")
        )
    except Exception as e:  # profiling is best-effort
        print("ntff hook setup failed:", e)


def _router(tensor_f32, Wr, br):
    """Return (topv, topi) exactly as the reference computes them (jax on CPU)."""
    try:
        import jax
        import jax.numpy as jnp

        cpu = jax.devices("cpu")[0]
        with jax.default_device(cpu):
            t = jax.device_put(jnp.asarray(tensor_f32), cpu)
            w = jax.device_put(jnp.asarray(Wr), cpu)
            b = jax.device_put(jnp.asarray(br), cpu)
            logits = jnp.einsum("bsd,de->bse", t, w) + b
            probs = jax.nn.softmax(logits, axis=-1)
            topv, topi = jax.lax.top_k(probs, NUM_SELECTED)
        return np.asarray(topv), np.asarray(topi)
    except Exception:
        # numpy fallback replicating jax semantics (stable ties by lower index)
        logits = (
            tensor_f32.reshape(-1, D_MODEL) @ np.asarray(Wr, np.float32)
        ) + np.asarray(br, np.float32)
        logits = logits.reshape(tensor_f32.shape[0], tensor_f32.shape[1], NUM_BANKS)
        m = logits.max(axis=-1, keepdims=True)
        e = np.exp(logits - m)
        probs = e / e.sum(axis=-1, keepdims=True)
        order = np.argsort(-probs, axis=-1, kind="stable")
        topi = order[..., :NUM_SELECTED]
        topv = np.take_along_axis(probs, topi, axis=-1)
        return topv.astype(np.float32), topi.astype(np.int32)


def _chunks_for(C, first_small=0):
    """Split capacity C into near-even matmul free-dim chunks <= 512,
    optionally with a small leading chunk."""
    if first_small and C > 2 * first_small:
        rest = C - first_small
        n = (rest + 511) // 512
        base = rest // n
        rem = rest - base * n
        return [first_small] + [base + (1 if i < rem else 0) for i in range(n)]
    n = (C + 511) // 512
    base = C // n
    rem = C - base * n
    return [base + (1 if i < rem else 0) for i in range(n)]


def _build_program(caps, chunks_list):
    import concourse.mybir as mybir
    import concourse.tile as tile
    from concourse import bacc

    wdt = mybir.dt.float16
    f32 = mybir.dt.float32
    nc = bacc.Bacc("TRN2", target_bir_lowering=False, debug=False, num_devices=N_CORES)

    xts = [
        nc.dram_tensor(f"xt{s}", [KD, P, caps[s]], wdt, kind="ExternalInput").ap()
        for s in range(N_SLOTS)
    ]
    w1s = [
        nc.dram_tensor(f"w1_{s}", [HH, P, KD, P], wdt, kind="ExternalInput").ap()
        for s in range(N_SLOTS)
    ]
    w2s = [
        nc.dram_tensor(f"w2_{s}", [MO, P, HH, P], wdt, kind="ExternalInput").ap()
        for s in range(N_SLOTS)
    ]
    b1s = [
        nc.dram_tensor(f"b1_{s}", [P, HH], f32, kind="ExternalInput").ap()
        for s in range(N_SLOTS)
    ]
    yts = [
        nc.dram_tensor(f"yt{s}", [MO, P, caps[s]], wdt, kind="ExternalOutput").ap()
        for s in range(N_SLOTS)
    ]

    Add = mybir.AluOpType.add
    Max = mybir.AluOpType.max
    tcmax = max(tn for chunks in chunks_list for tn in chunks)

    with tile.TileContext(nc) as tc:
        with (
            tc.tile_pool(name="xp", bufs=1) as xp,
            tc.tile_pool(name="bp", bufs=1) as bp,
            tc.tile_pool(name="w1p", bufs=_W1BUFS) as w1p,
            tc.tile_pool(name="w1pa", bufs=_W1ABUFS) as w1pa,
            tc.tile_pool(name="w2p", bufs=3) as w2p,
            tc.tile_pool(name="htp", bufs=2) as htp,
            tc.tile_pool(name="yp", bufs=4) as yp,
            tc.tile_pool(name="ps1", bufs=4, space="PSUM") as ps1,
            tc.tile_pool(name="ps2", bufs=4, space="PSUM") as ps2,
        ):
            # x tiles per (slot, k, chunk): contiguous DMAs, fine-grained deps.
            # DMA issue spread: w1 on sync; slot-0 x (head-critical; scalar
            # issues descriptors fastest) + w2 + y on scalar; b1 + later
            # slots' x on gpsimd so they never queue behind the w2 stream.
            xsb = {}

            def load_x(s, eng):
                t0 = 0
                for ci, tn in enumerate(chunks_list[s]):
                    for k in range(KD):
                        # With _XGATE the tag is slot-independent and the pool
                        # has bufs=1, so slot s+1's DMA carries a WAR dep on
                        # slot s's fc1 reads: the transfer starts only once
                        # fc1_s is done, i.e. inside the fc2_s window, instead
                        # of contending with the slot-s ramp.
                        tag = f"x_{k}_{ci}" if _XGATE else f"x_{s}_{k}_{ci}"
                        t = xp.tile([P, tn], wdt, tag=tag, name=f"x_{s}_{k}_{ci}")
                        eng.dma_start(t[:], xts[s][k, :, t0 : t0 + tn])
                        xsb[(s, k, ci)] = t
                    t0 += tn

            load_x(0, nc.scalar)
            b1sb = {}
            for s in range(N_SLOTS):
                b = bp.tile([P, HH], f32, tag=f"b1_{s}")
                nc.gpsimd.dma_start(b[:], b1s[s])
                b1sb[s] = b

            # PE warm-up: dummy matmuls on a memset tile. They depend on no
            # DMA, so the PE starts ~6us in and the DVFS p-state ramps while
            # the first x/w1 tiles are still in flight.
            if _WARM:
                wn = min(512, tcmax)
                warm = xp.tile([P, wn], wdt, tag="warm")
                nc.vector.memset(warm[:], 0.0)
                wps = ps1.tile([P, tcmax], f32, tag="ps1", name="warm_ps")
                for i in range(_WARM):
                    nc.tensor.matmul(
                        wps[:, :wn], warm[:, :128], warm[:], start=True, stop=True
                    )

            for s in range(N_SLOTS):
                chunks = chunks_list[s]
                starts = [sum(chunks[:i]) for i in range(len(chunks))]
                if not _XLATE and s + 1 < N_SLOTS:
                    load_x(s + 1, nc.scalar)

                ht = htp.tile([P, HH, caps[s]], wdt, tag="ht", name=f"ht_{s}")

                # fc1: per hm, run all chunks back-to-back (keeps the w1
                # consumption rate at the sustainable ~1 tile / 1.6-1.9us).
                for hm in range(HH):
                    pool = w1pa if s == 0 else w1p
                    w1sb = pool.tile(
                        [P, KD, P], wdt,
                        tag="w1a" if s == 0 else "w1",
                        name=f"w1_{s}_{hm}",
                    )
                    nc.sync.dma_start(w1sb[:], w1s[s][hm])
                    for ci, tn in enumerate(chunks):
                        t0 = starts[ci]
                        ps = ps1.tile(
                            [P, tcmax], f32, tag="ps1", name=f"ps1_{s}_{hm}_{ci}"
                        )
                        for k in range(KD):
                            nc.tensor.matmul(
                                ps[:, :tn],
                                w1sb[:, k],
                                xsb[(s, k, ci)][:, :tn],
                                start=(k == 0),
                                stop=(k == KD - 1),
                            )
                        # relu(psum + b1) on DVE, writes fp16 ht
                        nc.vector.tensor_scalar(
                            ht[:, hm, t0 : t0 + tn],
                            ps[:, :tn],
                            b1sb[s][:, hm : hm + 1],
                            0.0,
                            Add,
                            Max,
                        )

                # First two w2 tiles are issued ahead of the next slot's x so
                # the fc2 start never waits; the rest stream during fc2.
                w2sbs = {}
                if _W2PF:
                    for mo in range(min(_W2PF, MO)):
                        w2sb = w2p.tile(
                            [P, HH, P], wdt, tag="w2", name=f"w2_{s}_{mo}"
                        )
                        nc.scalar.dma_start(w2sb[:], w2s[s][mo])
                        w2sbs[mo] = w2sb

                # Next slot's x streams in during this slot's fc2 (issuing it
                # earlier would contend with the head-critical slot-0 stream).
                if _XLATE and s + 1 < N_SLOTS:
                    load_x(s + 1, nc.scalar)

                # fc2: partial output over this half's 16 k2 subtiles; no bias
                # (host adds b2 once per bank during the combine).
                for mo in range(MO):
                    if mo in w2sbs:
                        w2sb = w2sbs[mo]
                    else:
                        w2sb = w2p.tile([P, HH, P], wdt, tag="w2", name=f"w2_{s}_{mo}")
                        nc.scalar.dma_start(w2sb[:], w2s[s][mo])
                    pss = [
                        ps2.tile([P, tcmax], f32, tag="ps2", name=f"ps2_{s}_{mo}_{ci}")
                        for ci in range(len(chunks))
                    ]
                    for k2 in range(HH):
                        for ci, tn in enumerate(chunks):
                            nc.tensor.matmul(
                                pss[ci][:, :tn],
                                w2sb[:, k2],
                                ht[:, k2, starts[ci] : starts[ci] + tn],
                                start=(k2 == 0),
                                stop=(k2 == HH - 1),
                            )
                    for ci, tn in enumerate(chunks):
                        t0 = starts[ci]
                        ysb = yp.tile([P, tcmax], wdt, tag="y")
                        nc.vector.tensor_copy(ysb[:, :tn], pss[ci][:, :tn])
                        nc.sync.dma_start(yts[s][mo, :, t0 : t0 + tn], ysb[:, :tn])

    nc.compile()
    return nc


def kernel(tensor, Wr, br, W1, b1, W2, b2):
    global last_exec_time_ns, last_results
    from concourse import bass_utils

    t_np = np.asarray(tensor, np.float32)
    B, S, _ = t_np.shape
    T = B * S
    x = np.ascontiguousarray(t_np.reshape(T, D_MODEL))

    topv, topi = _router(t_np, np.asarray(Wr, np.float32), np.asarray(br, np.float32))
    topv = topv.reshape(T, NUM_SELECTED)
    topi = topi.reshape(T, NUM_SELECTED)

    # Per-bank token index lists + gates
    idx = []
    gates = []
    for e in range(NUM_BANKS):
        sel = np.nonzero((topi == e).any(axis=1))[0]
        idx.append(sel)
        g = np.where(topi[sel, 0] == e, topv[sel, 0], topv[sel, 1])
        gates.append(g.astype(np.float32))

    counts = np.array([len(i) for i in idx])

    # 32 half-pieces (bank, half), sorted by count desc; groups of 8 share a
    # slot; group capacity = max count in group, rounded up to even.
    pieces = [(e, h) for e in range(NUM_BANKS) for h in range(_SPLIT)]
    pieces.sort(key=lambda p: -counts[p[0]])
    groups = [pieces[g * N_CORES : (g + 1) * N_CORES] for g in range(N_SLOTS)]
    # Ascending capacity order: slot 0 (smallest) is single-chunk, so the
    # cold-start x demand is minimal and the first psum group can consume x
    # k-tiles as they land.
    if _ASC:
        groups = groups[::-1]
    caps = [max(max(int(counts[e]) for e, _ in grp), 2) for grp in groups]
    caps = [(c + 1) & ~1 for c in caps]
    chunks_list = [
        _chunks_for(caps[s], _FSMALL if s == 0 else 0) for s in range(N_SLOTS)
    ]

    np_wdt = np.float16

    # Gathered, feature-major tokens per (core, slot): [KD, P, cap]
    xt = [np.zeros((N_CORES, KD, P, caps[s]), dtype=np_wdt) for s in range(N_SLOTS)]
    for s in range(N_SLOTS):
        for c in range(N_CORES):
            e, _h = groups[s][c]
            n_e = counts[e]
            if n_e:
                xe_t = x[idx[e]].T.astype(np_wdt)  # [D_MODEL, n_e]
                xt[s][c, :, :, :n_e] = xe_t.reshape(KD, P, n_e)

    W1_np = np.asarray(W1, np.float32)
    W2_np = np.asarray(W2, np.float32)
    # w1d[e]: [MH, P(k-in-tile), KD, P(hm-col)]; half h -> rows h*HH:(h+1)*HH
    w1d = np.ascontiguousarray(
        W1_np.reshape(NUM_BANKS, KD, P, MH, P).transpose(0, 3, 2, 1, 4).astype(np_wdt)
    )
    # w2d[e]: [MO, P(k2-in-tile), MH, P(m-col)]; half h -> k2 h*HH:(h+1)*HH
    w2d = np.ascontiguousarray(
        W2_np.reshape(NUM_BANKS, MH, P, MO, P).transpose(0, 3, 2, 1, 4).astype(np_wdt)
    )
    b1d = np.ascontiguousarray(
        np.asarray(b1, np.float32).reshape(NUM_BANKS, MH, P).transpose(0, 2, 1)
    )
    b2_np = np.asarray(b2, np.float32)  # [NUM_BANKS, D_MODEL], host-side

    nc = _build_program(caps, chunks_list)

    in_maps = []
    for c in range(N_CORES):
        m = {}
        for s in range(N_SLOTS):
            e, h = groups[s][c]
            m[f"xt{s}"] = xt[s][c]
            m[f"w1_{s}"] = np.ascontiguousarray(w1d[e, h * HH : (h + 1) * HH])
            m[f"w2_{s}"] = np.ascontiguousarray(w2d[e][:, :, h * HH : (h + 1) * HH])
            m[f"b1_{s}"] = np.ascontiguousarray(b1d[e][:, h * HH : (h + 1) * HH])
        in_maps.append(m)

    if _PROFILE:
        _install_ntff_hook()
    res = bass_utils.run_bass_kernel_spmd(
        nc, in_maps, core_ids=list(range(N_CORES)), trace=_PROFILE
    )
    last_exec_time_ns = res.exec_time_ns
    last_results = res
    for _ in range(int(os.environ.get("BANKED_RERUNS", "0"))):
        r2 = bass_utils.run_bass_kernel_spmd(
            nc, in_maps, core_ids=list(range(N_CORES)), trace=_PROFILE
        )
        print("rerun exec_time_ns:", r2.exec_time_ns)

    # Host combine: sum half-piece partials per bank, add b2, gate, scatter.
    ybank = [None] * NUM_BANKS
    for s in range(N_SLOTS):
        for c in range(N_CORES):
            e, _h = groups[s][c]
            n_e = counts[e]
            if not n_e:
                continue
            ytc = res.results[c][f"yt{s}"]  # [MO, P, caps[s]] fp16
            ye = ytc.reshape(D_MODEL, caps[s])[:, :n_e].astype(np.float32)
            if ybank[e] is None:
                ybank[e] = ye.copy()
            else:
                ybank[e] += ye
    out = np.zeros((T, D_MODEL), dtype=np.float32)
    for e in range(NUM_BANKS):
        if counts[e]:
            ye = ybank[e] + b2_np[e][:, None]
            out[idx[e]] += gates[e][:, None] * ye.T
    return out.reshape(B, S, D_MODEL)


# revision 32
# speedup vs baseline: 1.0306x; 1.0306x over previous
# Banked (MoE top-2) feedforward on 8 TRN2 NeuronCores.
#
# Strategy (expert-parallel with hidden-dim splitting for load balance):
#   - Router (tiny: [T,1024]@[1024,16]) runs on host with jax-CPU, matching the
#     reference's einsum/softmax/top_k numerics so bank selection is identical.
#   - Work unit = half a bank: (bank e, half h) owns 16 of the 32 hidden
#     subtiles. Each half-piece computes HT = relu(W1h^T X^T + b1h) over its
#     16 hidden subtiles and a PARTIAL YT = W2h^T HT (fc2 contracting only its
#     half of D_HIDDEN, all 8 output subtiles). The host sums the two halves
#     and adds b2 during the combine, so no bias pass is needed for fc2.
#   - 32 half-pieces are sorted by token count and grouped 8-at-a-time into 4
#     per-core slots with graded capacities (cap = max count in group), which
#     cuts padded columns from ~1074 to ~1050 per core vs whole-bank slots.
#   - All matmuls in fp16 (full-rate PE, 2-byte weight loads); psum accumulates
#     fp32; y written back as fp16 (halves writeback traffic; host combines in
#     fp32). fp8 was measured at 4.3e-2 rel err -- over the gate -- so fp16.
#   - Head optimization: slot 0's first chunk is small so the first matmul only
#     waits for ~0.5MB of DMA; per-hm chunk interleave keeps the w1 stream rate
#     sustainable (~150GB/s) instead of bursting.

import os
import sys
import types

import numpy as np

# The device kernel runs through jax's axon PJRT backend. If the caller's
# environment pins JAX_PLATFORMS (e.g. to "cpu" for the reference), extend it
# so the axon backend stays reachable. Must happen before jax's first import.
_jp = os.environ.get("JAX_PLATFORMS", "")
if _jp and "axon" not in _jp.split(","):
    os.environ["JAX_PLATFORMS"] = _jp + ",axon"

D_MODEL = 1024
D_HIDDEN = 4096
NUM_BANKS = 16
NUM_SELECTED = 2
N_CORES = 8
P = 128
KD = D_MODEL // P     # 8  contraction subtiles for fc1
MH = D_HIDDEN // P    # 32 hidden subtiles total
_SPLIT = int(os.environ.get("BANKED_SPLIT", "2"))  # hidden pieces per bank
HH = MH // _SPLIT     # hidden subtiles per piece
MO = D_MODEL // P     # 8  output subtiles for fc2
N_SLOTS = NUM_BANKS * _SPLIT // N_CORES  # pieces per core

_PROFILE = bool(int(os.environ.get("BANKED_PROFILE", "0")))
_ASC = bool(int(os.environ.get("BANKED_ASC", "0")))      # slot order ascending
_WARM = int(os.environ.get("BANKED_WARM", "9"))          # warmup matmuls
_W2PF = int(os.environ.get("BANKED_W2PF", "0"))          # w2 tiles pre-issued
_XGATE = bool(int(os.environ.get("BANKED_XGATE", "0")))  # gate x_{s+1} on fc1_s
_XLATE = bool(int(os.environ.get("BANKED_XLATE", "0")))  # x_{s+1} after fc1
_FSMALL = int(os.environ.get("BANKED_FSMALL", "320"))    # small first chunk
_W1BUFS = int(os.environ.get("BANKED_W1BUFS", "6"))      # w1 prefetch depth
_W1ABUFS = int(os.environ.get("BANKED_W1ABUFS", "6"))    # slot-0 w1 depth
_W2BUFS = int(os.environ.get("BANKED_W2BUFS", "4"))      # w2 prefetch depth

last_exec_time_ns = None
last_results = None


def _install_ntff_hook():
    """bass_utils' axon trace path imports antenv.axon_hooks, which this image
    lacks; shim it and register the ctypes-based NTFF hook."""
    if "antenv.axon_hooks" in sys.modules:
        return
    mod = types.ModuleType("antenv.axon_hooks")
    mod._hook = None
    mod.set_axon_ntff_profile_hook = lambda h: setattr(mod, "_hook", h)
    mod.get_axon_ntff_profile_hook = lambda: mod._hook
    sys.modules["antenv.axon_hooks"] = mod
    try:
        from trn_agent_boot.trn_boot import _ntff_profile_via_ctypes

        mod.set_axon_ntff_profile_hook(
            _ntff_profile_via_ctypes("/opt/axon/libaxon_pjrt.so")
        )
    except Exception as e:  # profiling is best-effort
        print("ntff hook setup failed:", e)


def _router(tensor_f32, Wr, br):
    """Return (topv, topi) exactly as the reference computes them (jax on CPU)."""
    try:
        import jax
        import jax.numpy as jnp

        cpu = jax.devices("cpu")[0]
        with jax.default_device(cpu):
            t = jax.device_put(jnp.asarray(tensor_f32), cpu)
            w = jax.device_put(jnp.asarray(Wr), cpu)
            b = jax.device_put(jnp.asarray(br), cpu)
            logits = jnp.einsum("bsd,de->bse", t, w) + b
            probs = jax.nn.softmax(logits, axis=-1)
            topv, topi = jax.lax.top_k(probs, NUM_SELECTED)
        return np.asarray(topv), np.asarray(topi)
    except Exception:
        # numpy fallback replicating jax semantics (stable ties by lower index)
        logits = (
            tensor_f32.reshape(-1, D_MODEL) @ np.asarray(Wr, np.float32)
        ) + np.asarray(br, np.float32)
        logits = logits.reshape(tensor_f32.shape[0], tensor_f32.shape[1], NUM_BANKS)
        m = logits.max(axis=-1, keepdims=True)
        e = np.exp(logits - m)
        probs = e / e.sum(axis=-1, keepdims=True)
        order = np.argsort(-probs, axis=-1, kind="stable")
        topi = order[..., :NUM_SELECTED]
        topv = np.take_along_axis(probs, topi, axis=-1)
        return topv.astype(np.float32), topi.astype(np.int32)


def _chunks_for(C, first_small=0):
    """Split capacity C into near-even matmul free-dim chunks <= 512,
    optionally with a small leading chunk."""
    if first_small and C > 2 * first_small:
        rest = C - first_small
        n = (rest + 511) // 512
        base = rest // n
        rem = rest - base * n
        return [first_small] + [base + (1 if i < rem else 0) for i in range(n)]
    n = (C + 511) // 512
    base = C // n
    rem = C - base * n
    return [base + (1 if i < rem else 0) for i in range(n)]


def _build_program(caps, chunks_list):
    import concourse.mybir as mybir
    import concourse.tile as tile
    from concourse import bacc

    wdt = mybir.dt.float16
    f32 = mybir.dt.float32
    nc = bacc.Bacc("TRN2", target_bir_lowering=False, debug=False, num_devices=N_CORES)

    xts = [
        nc.dram_tensor(f"xt{s}", [KD, P, caps[s]], wdt, kind="ExternalInput").ap()
        for s in range(N_SLOTS)
    ]
    w1s = [
        nc.dram_tensor(f"w1_{s}", [HH, P, KD, P], wdt, kind="ExternalInput").ap()
        for s in range(N_SLOTS)
    ]
    w2s = [
        nc.dram_tensor(f"w2_{s}", [MO, P, HH, P], wdt, kind="ExternalInput").ap()
        for s in range(N_SLOTS)
    ]
    b1s = [
        nc.dram_tensor(f"b1_{s}", [P, HH], f32, kind="ExternalInput").ap()
        for s in range(N_SLOTS)
    ]
    yts = [
        nc.dram_tensor(f"yt{s}", [MO, P, caps[s]], wdt, kind="ExternalOutput").ap()
        for s in range(N_SLOTS)
    ]

    Add = mybir.AluOpType.add
    Max = mybir.AluOpType.max
    tcmax = max(tn for chunks in chunks_list for tn in chunks)

    with tile.TileContext(nc) as tc:
        with (
            tc.tile_pool(name="xp", bufs=1) as xp,
            tc.tile_pool(name="bp", bufs=1) as bp,
            tc.tile_pool(name="w1p", bufs=_W1BUFS) as w1p,
            tc.tile_pool(name="w1pa", bufs=_W1ABUFS) as w1pa,
            tc.tile_pool(name="w2p", bufs=_W2BUFS) as w2p,
            tc.tile_pool(name="htp", bufs=2) as htp,
            tc.tile_pool(name="yp", bufs=4) as yp,
            tc.tile_pool(name="ps1", bufs=4, space="PSUM") as ps1,
            tc.tile_pool(name="ps2", bufs=4, space="PSUM") as ps2,
        ):
            # x tiles per (slot, k, chunk): contiguous DMAs, fine-grained deps.
            # DMA issue spread: w1 on sync; slot-0 x (head-critical; scalar
            # issues descriptors fastest) + w2 + y on scalar; b1 + later
            # slots' x on gpsimd so they never queue behind the w2 stream.
            xsb = {}

            def load_x(s, eng):
                t0 = 0
                for ci, tn in enumerate(chunks_list[s]):
                    for k in range(KD):
                        # With _XGATE the tag is slot-independent and the pool
                        # has bufs=1, so slot s+1's DMA carries a WAR dep on
                        # slot s's fc1 reads: the transfer starts only once
                        # fc1_s is done, i.e. inside the fc2_s window, instead
                        # of contending with the slot-s ramp.
                        tag = f"x_{k}_{ci}" if _XGATE else f"x_{s}_{k}_{ci}"
                        t = xp.tile([P, tn], wdt, tag=tag, name=f"x_{s}_{k}_{ci}")
                        eng.dma_start(t[:], xts[s][k, :, t0 : t0 + tn])
                        xsb[(s, k, ci)] = t
                    t0 += tn

            load_x(0, nc.scalar)
            b1sb = {}
            for s in range(N_SLOTS):
                b = bp.tile([P, HH], f32, tag=f"b1_{s}")
                nc.gpsimd.dma_start(b[:], b1s[s])
                b1sb[s] = b

            # PE warm-up: dummy matmuls on a memset tile. They depend on no
            # DMA, so the PE starts ~6us in and the DVFS p-state ramps while
            # the first x/w1 tiles are still in flight.
            if _WARM:
                wn = min(512, tcmax)
                warm = xp.tile([P, wn], wdt, tag="warm")
                nc.vector.memset(warm[:], 0.0)
                wps = ps1.tile([P, tcmax], f32, tag="ps1", name="warm_ps")
                for i in range(_WARM):
                    nc.tensor.matmul(
                        wps[:, :wn], warm[:, :128], warm[:], start=True, stop=True
                    )

            for s in range(N_SLOTS):
                chunks = chunks_list[s]
                starts = [sum(chunks[:i]) for i in range(len(chunks))]
                if not _XLATE and s + 1 < N_SLOTS:
                    load_x(s + 1, nc.scalar)

                ht = htp.tile([P, HH, caps[s]], wdt, tag="ht", name=f"ht_{s}")

                # fc1: per hm, run all chunks back-to-back (keeps the w1
                # consumption rate at the sustainable ~1 tile / 1.6-1.9us).
                for hm in range(HH):
                    pool = w1pa if s == 0 else w1p
                    w1sb = pool.tile(
                        [P, KD, P], wdt,
                        tag="w1a" if s == 0 else "w1",
                        name=f"w1_{s}_{hm}",
                    )
                    nc.sync.dma_start(w1sb[:], w1s[s][hm])
                    for ci, tn in enumerate(chunks):
                        t0 = starts[ci]
                        ps = ps1.tile(
                            [P, tcmax], f32, tag="ps1", name=f"ps1_{s}_{hm}_{ci}"
                        )
                        for k in range(KD):
                            nc.tensor.matmul(
                                ps[:, :tn],
                                w1sb[:, k],
                                xsb[(s, k, ci)][:, :tn],
                                start=(k == 0),
                                stop=(k == KD - 1),
                            )
                        # relu(psum + b1) on DVE, writes fp16 ht
                        nc.vector.tensor_scalar(
                            ht[:, hm, t0 : t0 + tn],
                            ps[:, :tn],
                            b1sb[s][:, hm : hm + 1],
                            0.0,
                            Add,
                            Max,
                        )

                # First two w2 tiles are issued ahead of the next slot's x so
                # the fc2 start never waits; the rest stream during fc2.
                w2sbs = {}
                if _W2PF:
                    for mo in range(min(_W2PF, MO)):
                        w2sb = w2p.tile(
                            [P, HH, P], wdt, tag="w2", name=f"w2_{s}_{mo}"
                        )
                        nc.scalar.dma_start(w2sb[:], w2s[s][mo])
                        w2sbs[mo] = w2sb

                # Next slot's x streams in during this slot's fc2 (issuing it
                # earlier would contend with the head-critical slot-0 stream).
                if _XLATE and s + 1 < N_SLOTS:
                    load_x(s + 1, nc.scalar)

                # fc2: partial output over this half's 16 k2 subtiles; no bias
                # (host adds b2 once per bank during the combine).
                for mo in range(MO):
                    if mo in w2sbs:
                        w2sb = w2sbs[mo]
                    else:
                        w2sb = w2p.tile([P, HH, P], wdt, tag="w2", name=f"w2_{s}_{mo}")
                        nc.scalar.dma_start(w2sb[:], w2s[s][mo])
                    pss = [
                        ps2.tile([P, tcmax], f32, tag="ps2", name=f"ps2_{s}_{mo}_{ci}")
                        for ci in range(len(chunks))
                    ]
                    for k2 in range(HH):
                        for ci, tn in enumerate(chunks):
                            nc.tensor.matmul(
                                pss[ci][:, :tn],
                                w2sb[:, k2],
                                ht[:, k2, starts[ci] : starts[ci] + tn],
                                start=(k2 == 0),
                                stop=(k2 == HH - 1),
                            )
                    for ci, tn in enumerate(chunks):
                        t0 = starts[ci]
                        ysb = yp.tile([P, tcmax], wdt, tag="y")
                        nc.vector.tensor_copy(ysb[:, :tn], pss[ci][:, :tn])
                        nc.sync.dma_start(yts[s][mo, :, t0 : t0 + tn], ysb[:, :tn])

    nc.compile()
    return nc


def kernel(tensor, Wr, br, W1, b1, W2, b2):
    global last_exec_time_ns, last_results
    from concourse import bass_utils

    t_np = np.asarray(tensor, np.float32)
    B, S, _ = t_np.shape
    T = B * S
    x = np.ascontiguousarray(t_np.reshape(T, D_MODEL))

    topv, topi = _router(t_np, np.asarray(Wr, np.float32), np.asarray(br, np.float32))
    topv = topv.reshape(T, NUM_SELECTED)
    topi = topi.reshape(T, NUM_SELECTED)

    # Per-bank token index lists + gates
    idx = []
    gates = []
    for e in range(NUM_BANKS):
        sel = np.nonzero((topi == e).any(axis=1))[0]
        idx.append(sel)
        g = np.where(topi[sel, 0] == e, topv[sel, 0], topv[sel, 1])
        gates.append(g.astype(np.float32))

    counts = np.array([len(i) for i in idx])

    # 32 half-pieces (bank, half), sorted by count desc; groups of 8 share a
    # slot; group capacity = max count in group, rounded up to even.
    pieces = [(e, h) for e in range(NUM_BANKS) for h in range(_SPLIT)]
    pieces.sort(key=lambda p: -counts[p[0]])
    groups = [pieces[g * N_CORES : (g + 1) * N_CORES] for g in range(N_SLOTS)]
    # Ascending capacity order: slot 0 (smallest) is single-chunk, so the
    # cold-start x demand is minimal and the first psum group can consume x
    # k-tiles as they land.
    if _ASC:
        groups = groups[::-1]
    caps = [max(max(int(counts[e]) for e, _ in grp), 2) for grp in groups]
    caps = [(c + 1) & ~1 for c in caps]
    chunks_list = [
        _chunks_for(caps[s], _FSMALL if s == 0 else 0) for s in range(N_SLOTS)
    ]

    np_wdt = np.float16

    # Gathered, feature-major tokens per (core, slot): [KD, P, cap]
    xt = [np.zeros((N_CORES, KD, P, caps[s]), dtype=np_wdt) for s in range(N_SLOTS)]
    for s in range(N_SLOTS):
        for c in range(N_CORES):
            e, _h = groups[s][c]
            n_e = counts[e]
            if n_e:
                xe_t = x[idx[e]].T.astype(np_wdt)  # [D_MODEL, n_e]
                xt[s][c, :, :, :n_e] = xe_t.reshape(KD, P, n_e)

    W1_np = np.asarray(W1, np.float32)
    W2_np = np.asarray(W2, np.float32)
    # w1d[e]: [MH, P(k-in-tile), KD, P(hm-col)]; half h -> rows h*HH:(h+1)*HH
    w1d = np.ascontiguousarray(
        W1_np.reshape(NUM_BANKS, KD, P, MH, P).transpose(0, 3, 2, 1, 4).astype(np_wdt)
    )
    # w2d[e]: [MO, P(k2-in-tile), MH, P(m-col)]; half h -> k2 h*HH:(h+1)*HH
    w2d = np.ascontiguousarray(
        W2_np.reshape(NUM_BANKS, MH, P, MO, P).transpose(0, 3, 2, 1, 4).astype(np_wdt)
    )
    b1d = np.ascontiguousarray(
        np.asarray(b1, np.float32).reshape(NUM_BANKS, MH, P).transpose(0, 2, 1)
    )
    b2_np = np.asarray(b2, np.float32)  # [NUM_BANKS, D_MODEL], host-side

    nc = _build_program(caps, chunks_list)

    in_maps = []
    for c in range(N_CORES):
        m = {}
        for s in range(N_SLOTS):
            e, h = groups[s][c]
            m[f"xt{s}"] = xt[s][c]
            m[f"w1_{s}"] = np.ascontiguousarray(w1d[e, h * HH : (h + 1) * HH])
            m[f"w2_{s}"] = np.ascontiguousarray(w2d[e][:, :, h * HH : (h + 1) * HH])
            m[f"b1_{s}"] = np.ascontiguousarray(b1d[e][:, h * HH : (h + 1) * HH])
        in_maps.append(m)

    if _PROFILE:
        _install_ntff_hook()
    res = bass_utils.run_bass_kernel_spmd(
        nc, in_maps, core_ids=list(range(N_CORES)), trace=_PROFILE
    )
    last_exec_time_ns = res.exec_time_ns
    last_results = res
    for _ in range(int(os.environ.get("BANKED_RERUNS", "0"))):
        r2 = bass_utils.run_bass_kernel_spmd(
            nc, in_maps, core_ids=list(range(N_CORES)), trace=_PROFILE
        )
        print("rerun exec_time_ns:", r2.exec_time_ns)

    # Host combine: sum half-piece partials per bank, add b2, gate, scatter.
    ybank = [None] * NUM_BANKS
    for s in range(N_SLOTS):
        for c in range(N_CORES):
            e, _h = groups[s][c]
            n_e = counts[e]
            if not n_e:
                continue
            ytc = res.results[c][f"yt{s}"]  # [MO, P, caps[s]] fp16
            ye = ytc.reshape(D_MODEL, caps[s])[:, :n_e].astype(np.float32)
            if ybank[e] is None:
                ybank[e] = ye.copy()
            else:
                ybank[e] += ye
    out = np.zeros((T, D_MODEL), dtype=np.float32)
    for e in range(NUM_BANKS):
        if counts[e]:
            ye = ybank[e] + b2_np[e][:, None]
            out[idx[e]] += gates[e][:, None] * ye.T
    return out.reshape(B, S, D_MODEL)
